# revision 41
# baseline (speedup 1.0000x reference)
"""AFDecoder Trainium2 kernel: gaussian splat + centered-FFT hartley.

Strategy: pure batch-parallel over 8 NeuronCores (8 images each).
Per image on device:
  - pose atoms (composite 3x3 passed from host as runtime data)
  - splat: banded-matrix formulation  img = Ry^T @ Cx  where
    Ry[n, r] = exp(-(r - cy_n)^2 / (2 s^2)) * [|r - round(cy_n)| <= 5].
    Band rows are produced by three load-balanced paths (engines run at
    ~300us each): "P" = indirect-DMA row gather from a host-precomputed
    (shift x 64-level sub-pixel offset) gaussian table; "M" = DVE
    subtract/clamp/square + ACT exp; "A" = ACT Square-with-bias + exp.
    PE contracts 128-atom chunks into PSUM (bf16 in, f32 accumulate).
  - hartley: f = A X A with A = S F S (shifted DFT);  y = Re - Im =
    (C@X) @ (C-S) - (S@X) @ (C+S), 4 real bf16 matmuls via PE.
Outputs y, y_real concatenated on host.
"""

import sys

for p in ("/opt/trn_rl_repo",):
    if p not in sys.path:
        sys.path.insert(0, p)

import numpy as np

import concourse.bass as bass
import concourse.bacc as bacc
import concourse.tile as tile
from concourse import mybir
from concourse.bass_utils import run_bass_kernel_spmd

D = 384
PIX = 1.0
SIGMA = 1.5
INV2S2 = 1.0 / (2.0 * SIGMA * SIGMA)
N_ATOMS = 8192
B_FULL = 64
N_CORES = 8
B_LOC = B_FULL // N_CORES  # 8 images per core
P = 128
NCH = N_ATOMS // P  # 64 atoms per partition
NMT = D // P  # 3 row tiles

F32 = mybir.dt.float32
F16 = mybir.dt.float16
BF16 = mybir.dt.bfloat16
I32 = mybir.dt.int32

NQ = 64        # delta quantization levels
SMIN = -6      # lowest useful round(c) after clamp
NS = 396       # number of integer shifts (icy in [-6, 389])
_CACHE = {}
LAST_EXEC_NS = None
LAST_RUN_WALL = None
TRACE = False


def _gauss_table() -> np.ndarray:
    # row (s*NQ + q): masked gaussian band for icy' = s + SMIN, delta bin q
    s = np.arange(NS)[:, None, None]
    q = np.arange(NQ)[None, :, None]
    r = np.arange(D)[None, None, :]
    icy = s + SMIN
    delta = -0.5 + (q + 0.5) / NQ          # delta = round(c) - c
    cx = icy - delta                        # c = round(c) - delta
    val = np.exp(-((r - cx) ** 2) * INV2S2) * (np.abs(r - icy) <= 5)
    return val.reshape(NS * NQ, D).astype(np.float32)


def _dft_consts() -> np.ndarray:
    n = np.arange(D)
    F = np.exp(-2j * np.pi * np.outer(n, n) / D)
    Sh = np.zeros((D, D))
    Sh[n, (n + D // 2) % D] = 1.0
    A = Sh @ F @ Sh
    C = A.real
    S = A.imag
    # stage2 uses rhs = (C - S) and -(C + S) so PSUM accumulation is all adds
    return np.stack([C, S, C - S, -(C + S)]).astype(np.float32)


def _build_graph() -> bass.Bass:
    nc = bacc.Bacc("TRN2", target_bir_lowering=False)
    crd_p = nc.declare_dram_parameter("crd", [B_LOC, N_ATOMS, 3], F32, isOutput=False)
    pose_p = nc.declare_dram_parameter("pose", [P, B_LOC * 16], F32, isOutput=False)
    dft_p = nc.declare_dram_parameter("dft", [4, D, D], BF16, isOutput=False)
    gt_p = nc.declare_dram_parameter("gtab", [NS * NQ, D], BF16, isOutput=False)
    y_p = nc.declare_dram_parameter("y", [B_LOC, D, D], F32, isOutput=True)
    yr_p = nc.declare_dram_parameter("yreal", [B_LOC, D, D], F32, isOutput=True)

    from contextlib import ExitStack

    with ExitStack() as es:
        tc = es.enter_context(tile.TileContext(nc))
        cpool = es.enter_context(tc.tile_pool(name="consts", bufs=1))
        wpool = es.enter_context(tc.tile_pool(name="work", bufs=2))
        bpool = es.enter_context(tc.tile_pool(name="band", bufs=4))
        spool = es.enter_context(tc.tile_pool(name="stage", bufs=4))
        px = es.enter_context(tc.tile_pool(name="px", bufs=2, space="PSUM"))
        pt = es.enter_context(tc.tile_pool(name="pt", bufs=2, space="PSUM"))

        # ---- constants ----
        rowidx_i = cpool.tile([P, D], I32, tag="rowidx_i")
        nc.gpsimd.iota(rowidx_i[:], pattern=[[1, D]], base=0, channel_multiplier=0)
        rowidx = cpool.tile([P, D], F16, tag="rowidx")
        nc.vector.tensor_copy(out=rowidx[:], in_=rowidx_i[:])

        # DFT matrices as bf16, chunked [P, D] along rows; A symmetric so
        # chunk r of rows == chunk r of cols.
        dft_sb = []  # [4][NMT] tiles
        for m in range(4):
            row = []
            for r in range(NMT):
                tl = cpool.tile([P, D], BF16, tag=f"dftA{m}{r}", name=f"dftA{m}{r}")
                nc.sync.dma_start(out=tl[:], in_=dft_p[m, r * P : (r + 1) * P, :])
                tb = cpool.tile([P, D], BF16, tag=f"dft{m}{r}", name=f"dft{m}{r}")
                nc.vector.tensor_copy(out=tb[:], in_=tl[:])
                row.append(tb)
            dft_sb.append(row)
        Cc, Sc, Mm, Mpn = dft_sb

        pose_ld = cpool.tile([P, B_LOC * 16], F32, tag="pose_ld")
        nc.sync.dma_start(out=pose_ld[:], in_=pose_p[:, :])
        pose_bc = cpool.tile([P, B_LOC * 16], F32, tag="pose_bc")
        nc.vector.tensor_copy(out=pose_bc[:], in_=pose_ld[:])

        def psc(b, k):
            # pose scalar broadcast across partitions: [P, 1] AP
            return pose_bc[:, 16 * b + k : 16 * b + k + 1]

        for b in range(B_LOC):
            # ---- load + pose ----
            crd_ld = cpool.tile([P, NCH * 3], F32, tag=f"crd_ld{b}", name=f"crd_ld{b}")
            nc.sync.dma_start(
                out=crd_ld[:],
                in_=crd_p[b].rearrange("(p j) c -> p (j c)", p=P),
            )
            crd_t = wpool.tile([P, NCH * 3], F32, tag="crd")
            nc.vector.tensor_copy(out=crd_t[:], in_=crd_ld[:])
            c3 = crd_t[:].rearrange("p (j c) -> p j c", c=3)
            xs, ys, zs = c3[:, :, 0], c3[:, :, 1], c3[:, :, 2]

            coord = {}
            for ax in (0, 1):  # 0=x (cols), 1=y (rows)
                k0 = 4 * ax
                cc = wpool.tile([P, NCH], F32, tag=f"cc{ax}")
                tmp = wpool.tile([P, NCH], F32, tag=f"ctmp{ax}")
                nc.vector.tensor_scalar(
                    out=cc[:], in0=xs, scalar1=psc(b, k0 + 0), scalar2=None,
                    op0=mybir.AluOpType.mult,
                )
                nc.vector.tensor_scalar(
                    out=tmp[:], in0=ys, scalar1=psc(b, k0 + 1), scalar2=None,
                    op0=mybir.AluOpType.mult,
                )
                nc.vector.tensor_tensor(
                    out=cc[:], in0=cc[:], in1=tmp[:], op=mybir.AluOpType.add
                )
                nc.vector.tensor_scalar(
                    out=tmp[:], in0=zs, scalar1=psc(b, k0 + 2), scalar2=None,
                    op0=mybir.AluOpType.mult,
                )
                nc.vector.tensor_tensor(
                    out=cc[:], in0=cc[:], in1=tmp[:], op=mybir.AluOpType.add
                )
                # + trans + D/2  (pixel-space center coordinate)
                nc.vector.tensor_scalar(
                    out=cc[:], in0=cc[:], scalar1=psc(b, k0 + 3), scalar2=float(D // 2),
                    op0=mybir.AluOpType.add, op1=mybir.AluOpType.add,
                )
                ici = wpool.tile([P, NCH], I32, tag=f"ici{ax}")
                nc.vector.tensor_copy(out=ici[:], in_=cc[:])  # f32->i32 rnd
                icf = wpool.tile([P, NCH], F32, tag=f"icf{ax}")
                nc.vector.tensor_copy(out=icf[:], in_=ici[:])
                dd = wpool.tile([P, NCH], F32, tag=f"dd{ax}")
                nc.vector.tensor_tensor(
                    out=dd[:], in0=icf[:], in1=cc[:], op=mybir.AluOpType.subtract
                )
                # table row index: (clamp(ic,-6,389)+6)*NQ + clamp(round(dd*NQ+NQ/2-0.5),0,NQ-1)
                sf = wpool.tile([P, NCH], F32, tag=f"sf{ax}")
                nc.vector.tensor_scalar(
                    out=sf[:], in0=icf[:], scalar1=float(SMIN), scalar2=float(SMIN + NS - 1),
                    op0=mybir.AluOpType.max, op1=mybir.AluOpType.min,
                )
                qf = wpool.tile([P, NCH], F32, tag=f"qf{ax}")
                nc.vector.tensor_scalar(
                    out=qf[:], in0=dd[:], scalar1=float(NQ), scalar2=NQ / 2 - 0.5,
                    op0=mybir.AluOpType.mult, op1=mybir.AluOpType.add,
                )
                qf2 = wpool.tile([P, NCH], F32, tag=f"qf2{ax}")
                nc.vector.tensor_scalar(
                    out=qf2[:], in0=qf[:], scalar1=0.0, scalar2=float(NQ - 1),
                    op0=mybir.AluOpType.max, op1=mybir.AluOpType.min,
                )
                sa = wpool.tile([P, NCH], F32, tag=f"sa{ax}")
                nc.vector.tensor_scalar(
                    out=sa[:], in0=sf[:], scalar1=float(NQ), scalar2=float(-SMIN * NQ),
                    op0=mybir.AluOpType.mult, op1=mybir.AluOpType.add,
                )
                idxf = wpool.tile([P, NCH], F32, tag=f"idxf{ax}")
                nc.vector.tensor_tensor(
                    out=idxf[:], in0=sa[:], in1=qf2[:], op=mybir.AluOpType.add
                )
                idxi = wpool.tile([P, NCH], I32, tag=f"idxi{ax}")
                nc.vector.tensor_copy(out=idxi[:], in_=idxf[:])
                negcc = wpool.tile([P, NCH], F32, tag=f"negcc{ax}")
                nc.vector.tensor_scalar(
                    out=negcc[:], in0=cc[:], scalar1=-1.0, scalar2=None,
                    op0=mybir.AluOpType.mult,
                )
                coord[ax] = (idxi, negcc, cc)

            # ---- splat: hybrid band build (Pool gather | DVE+ACT dense) ----
            psX = [px.tile([P, D], F32, space="PSUM", tag=f"X{m}", name=f"psX{m}") for m in range(NMT)]
            A_SET = set()
            for j in range(NCH):
                path = "M" if j % 2 == 0 else "P"
                bands = {}
                for ax in (1, 0):
                    idxi, negcc, cc_t = coord[ax]
                    if path == "P":
                        bt = bpool.tile([P, D], BF16, tag=f"band{ax}", name=f"band{ax}", bufs=8)
                        nc.gpsimd.indirect_dma_start(
                            out=bt[:],
                            out_offset=None,
                            in_=gt_p[:],
                            in_offset=bass.IndirectOffsetOnAxis(
                                ap=idxi[:, j : j + 1], axis=0
                            ),
                        )
                    elif path == "A":
                        # unmasked gaussian, pure-ACT: sq=(r-c)^2 via Square
                        # bias, f32 so far atoms give exp(-huge)=0 not inf
                        sq = bpool.tile([P, D], F32, tag=f"sq{ax}", bufs=3)
                        nc.scalar.activation(
                            out=sq[:], in_=rowidx[:],
                            func=mybir.ActivationFunctionType.Square,
                            bias=negcc[:, j : j + 1],
                        )
                        bt = bpool.tile([P, D], BF16, tag=f"bandD{ax}", bufs=4)
                        nc.scalar.activation(
                            out=bt[:], in_=sq[:],
                            func=mybir.ActivationFunctionType.Exp, scale=-INV2S2,
                        )
                    else:
                        # unmasked gaussian, DVE square + ACT exp; sq in bf16
                        # (f32 exponent range) so far atoms overflow-free ->
                        # exp(-huge)=0, no clamp needed
                        tp = bpool.tile([P, D], F16, tag=f"tp{ax}", bufs=3)
                        nc.vector.tensor_scalar(
                            out=tp[:], in0=rowidx[:], scalar1=cc_t[:, j : j + 1],
                            scalar2=None, op0=mybir.AluOpType.subtract,
                        )
                        sqm = bpool.tile([P, D], BF16, tag=f"sqm{ax}", bufs=3)
                        nc.vector.tensor_tensor(
                            out=sqm[:], in0=tp[:], in1=tp[:],
                            op=mybir.AluOpType.mult,
                        )
                        bt = bpool.tile([P, D], BF16, tag=f"bandM{ax}", bufs=4)
                        nc.scalar.activation(
                            out=bt[:], in_=sqm[:],
                            func=mybir.ActivationFunctionType.Exp, scale=-INV2S2,
                        )
                    bands[ax] = bt
                for m in range(NMT):
                    nc.tensor.matmul(
                        out=psX[m][:],
                        lhsT=bands[1][:, m * P : (m + 1) * P],
                        rhs=bands[0][:],
                        start=(j == 0),
                        stop=(j == NCH - 1),
                    )

            # ---- y_real out + bf16 copy for FFT ----
            Xbf = []
            for m in range(NMT):
                xf = spool.tile([P, D], F32, tag=f"Xf{m}", name=f"Xf{m}")
                nc.scalar.activation(
                    out=xf[:], in_=psX[m][:],
                    func=mybir.ActivationFunctionType.Copy,
                )
                nc.sync.dma_start(
                    out=yr_p[b, m * P : (m + 1) * P, :], in_=xf[:]
                )
                xb = spool.tile([P, D], BF16, tag=f"Xbf{m}")
                nc.vector.tensor_copy(out=xb[:], in_=psX[m][:])
                Xbf.append(xb)

            # ---- stage 1 (pre-transposed): T1^T = X^T @ C, T2^T = X^T @ S ----
            # lhsT = X (rows on partitions), rhs = C/S; out lands [c, k] so
            # stage 2 consumes it directly with no PE transposes.
            Tt = {}
            for which, MAT in (("T1", Cc), ("T2", Sc)):
                for ct in range(NMT):
                    ps = pt.tile([P, D], F32, space="PSUM", tag="pstage")
                    for r in range(NMT):
                        nc.tensor.matmul(
                            out=ps[:],
                            lhsT=Xbf[r][:, ct * P : (ct + 1) * P],
                            rhs=MAT[r][:],
                            start=(r == 0),
                            stop=(r == NMT - 1),
                        )
                    tt_ = spool.tile([P, D], BF16, tag=f"{which}t{ct}", name=f"{which}t{ct}")
                    nc.vector.tensor_copy(out=tt_[:], in_=ps[:])
                    Tt[(which, ct)] = tt_

            # ---- stage 2: y = T1 @ (C-S) - T2 @ (C+S)  (contract over c) ----
            for k in range(NMT):
                ps = pt.tile([P, D], F32, space="PSUM", tag="pstage")
                first = True
                for c in range(NMT):
                    nc.tensor.matmul(
                        out=ps[:], lhsT=Tt[("T1", c)][:, k * P : (k + 1) * P],
                        rhs=Mm[c][:], start=first, stop=False,
                    )
                    first = False
                    nc.tensor.matmul(
                        out=ps[:], lhsT=Tt[("T2", c)][:, k * P : (k + 1) * P],
                        rhs=Mpn[c][:], start=False, stop=(c == NMT - 1),
                    )
                yf = spool.tile([P, D], F32, tag="yf")
                nc.scalar.activation(
                    out=yf[:], in_=ps[:],
                    func=mybir.ActivationFunctionType.Copy,
                )
                nc.sync.dma_start(out=y_p[b, k * P : (k + 1) * P, :], in_=yf[:])

    nc.compile()
    return nc


def kernel(crd, rot, rot_init, trans_init):
    crd = np.asarray(crd, np.float32)
    rot = np.asarray(rot, np.float32)
    rot_init = np.asarray(rot_init, np.float32)
    trans_init = np.asarray(trans_init, np.float32)

    # composite pose: crd2 = crd @ comp + tb,  comp = rot_init @ rot_b^T
    comp = np.einsum("ij,bkj->bik", rot_init, rot)  # [B,3,3]
    tb = np.einsum("j,bkj->bk", trans_init, rot)  # [B,3]
    pose = np.zeros((B_FULL, 16), np.float32)
    pose[:, 0:3] = comp[:, :, 0]  # x' = crd . comp[:,0] + tb[0]
    pose[:, 3] = tb[:, 0]
    pose[:, 4:7] = comp[:, :, 1]
    pose[:, 7] = tb[:, 1]

    import ml_dtypes

    dft = _dft_consts().astype(ml_dtypes.bfloat16)
    gtab = _gauss_table().astype(ml_dtypes.bfloat16)

    if "nc" not in _CACHE:
        _CACHE["nc"] = _build_graph()
    nc = _CACHE["nc"]

    in_maps = [
        {
            "crd": np.ascontiguousarray(crd[c * B_LOC : (c + 1) * B_LOC]),
            "pose": np.ascontiguousarray(
                np.broadcast_to(
                    pose[c * B_LOC : (c + 1) * B_LOC].reshape(1, B_LOC * 16),
                    (P, B_LOC * 16),
                )
            ),
            "dft": dft,
            "gtab": gtab,
        }
        for c in range(N_CORES)
    ]
    global LAST_EXEC_NS, LAST_RUN_WALL
    import time as _time
    out = run_bass_kernel_spmd(nc, in_maps, list(range(N_CORES)))
    _t0 = _time.time()
    out = run_bass_kernel_spmd(nc, in_maps, list(range(N_CORES)))
    LAST_RUN_WALL = _time.time() - _t0
    LAST_EXEC_NS = out.exec_time_ns
    res = out.results
    y = np.concatenate([res[c]["y"] for c in range(N_CORES)], axis=0)
    yr = np.concatenate([res[c]["yreal"] for c in range(N_CORES)], axis=0)
    return y, yr
